# revision 33
# baseline (speedup 1.0000x reference)
"""Trainium2 Bass kernel for nn_CF_34016140984494 (dense_cnn).

Data-parallel over batch N=8 across 8 NeuronCores (1 image per core).
Per-core pipeline (restructured v2 for engine overlap):
  A: conv1(1x1)+BN1 -> nearest-upsample -> attention fuse  (per spatial tile)
  B: offset conv3x3 (per tile, 1-tile lag behind A)
  C: folded index/bilinear-weight pipeline [63,448]; weights broadcast to
     128 partitions via DRAM round-trip DMA (no PE/PSUM involved)
  D: ap_gather (d=2 pair-packed, full-image window) -> in-place lerp ->
     DCN matmul with strided x-pair rhs views (pair-sum folded into PSUM)
Returns (out, xf) like the reference.
"""
import numpy as np
from contextlib import ExitStack

N, CIN, MID, OUT, H, W = 8, 512, 256, 256, 56, 56
HY, WY = 28, 28
EPS = 1e-5
K = 9
HP, WP = 59, 58               # padded rows (1 top, 2 bottom zero), cols
PP = HP * WP                  # 3422
P = H * W                     # 3136
NT = 7
TS = P // NT                  # 448
FJ = 7
FR = K * FJ                   # 63
WRAP = TS // 16               # 28
GN = 3368                     # gather num_elems (covers idx<=3363)
XFBN = 3432                   # padded xfb alloc (>= GN+58+2, mult of 8)

_CACHE = {}


# ----------------------------------------------------------------- host math
def _fold_weights(d):
    f = {}
    scale1 = d['bn1_gamma'] / np.sqrt(d['bn1_var'] + EPS)
    bias1 = d['bn1_beta'] - d['bn1_mean'] * scale1
    W1 = (d['conv1_w'] * scale1[:, None]).astype(np.float32)            # [MID, CIN]
    f['w1T'] = np.ascontiguousarray(W1.T)                               # [CIN, MID]
    f['b1'] = bias1.astype(np.float32).reshape(MID, 1)
    f['axT'] = np.ascontiguousarray(d['att_w'][:, :MID].astype(np.float32).T)  # [MID, 2]
    f['ayT'] = np.ascontiguousarray(d['att_w'][:, MID:].astype(np.float32).T)  # [MID, 2]
    f['ba'] = d['att_b'].astype(np.float32).reshape(2, 1)
    ow = d['off_w'].reshape(2 * K, MID, K)
    perm = np.concatenate([np.arange(0, 18, 2), np.arange(1, 18, 2)])   # dy rows, then dx rows
    f['woffT'] = np.ascontiguousarray(ow[perm].transpose(1, 2, 0)).astype(np.float32)  # [MID, K, 18]
    scale2 = d['bn2_gamma'] / np.sqrt(d['bn2_var'] + EPS)
    W2 = (d['dcn_w'].reshape(OUT, MID * K) * scale2[:, None]).astype(np.float32)
    w2r = W2.reshape(OUT, MID, K).transpose(2, 1, 0).reshape(MID * K, OUT)  # rows k-major c-minor
    f['w2T'] = np.ascontiguousarray(w2r)
    f['b2'] = (d['bn2_beta'] - d['bn2_mean'] * scale2).astype(np.float32).reshape(OUT, 1)
    return f


def _build_consts():
    """Folded base-coordinate tables [63, 448]: row = k*7 + j, col = s."""
    kv = np.arange(K)
    ky = (kv // 3).astype(np.float32)
    kx = (kv % 3).astype(np.float32)
    p = np.arange(P)
    hh = (p // W).astype(np.float32)
    ww = (p % W).astype(np.float32)
    basey = hh[None, :] - 1.0 + ky[:, None]      # [9, P]
    basex = ww[None, :] - 1.0 + kx[:, None]
    byf = basey.reshape(K, FJ, TS).reshape(FR, TS)
    bxf = basex.reshape(K, FJ, TS).reshape(FR, TS)
    return byf.astype(np.float32), bxf.astype(np.float32)


# ------------------------------------------------------------- device build
def _build_program():
    import concourse.bass as bass
    import concourse.tile as tile
    from concourse import bacc, mybir
    dt = mybir.dt
    AF = mybir.ActivationFunctionType
    OP = mybir.AluOpType

    nc = bacc.Bacc("TRN2", target_bir_lowering=False, debug=False, num_devices=N)

    f32, bf16 = dt.float32, dt.bfloat16
    i16, i32 = dt.int16, dt.int32

    # --- DRAM I/O
    x_im = nc.dram_tensor("x_im", [CIN, P], bf16, kind="ExternalInput").ap()
    y_im = nc.dram_tensor("y_im", [MID, HY * WY], bf16, kind="ExternalInput").ap()
    w1T = nc.dram_tensor("w1T", [CIN, MID], bf16, kind="ExternalInput").ap()
    axT = nc.dram_tensor("axT", [MID, 2], bf16, kind="ExternalInput").ap()
    ayT = nc.dram_tensor("ayT", [MID, 2], bf16, kind="ExternalInput").ap()
    woffT = nc.dram_tensor("woffT", [MID, K, 18], bf16, kind="ExternalInput").ap()
    w2T = nc.dram_tensor("w2T", [MID * K, OUT], bf16, kind="ExternalInput").ap()
    b1 = nc.dram_tensor("b1", [MID, 1], f32, kind="ExternalInput").ap()
    ba = nc.dram_tensor("ba", [2, 1], f32, kind="ExternalInput").ap()
    b2 = nc.dram_tensor("b2", [OUT, 1], f32, kind="ExternalInput").ap()
    basey = nc.dram_tensor("basey", [FR, TS], f32, kind="ExternalInput").ap()
    basex = nc.dram_tensor("basex", [FR, TS], f32, kind="ExternalInput").ap()
    idm2 = nc.dram_tensor("idm2", [2, 2], f32, kind="ExternalInput").ap()
    out_im = nc.dram_tensor("out_im", [OUT, P], f32, kind="ExternalOutput").ap()
    xf_im = nc.dram_tensor("xf_im", [MID, P], f32, kind="ExternalOutput").ap()
    # internal DRAM scratch
    soff = nc.dram_tensor("soff", [18, P], f32).ap()
    wqd = nc.dram_tensor("wqd", [K, FJ, TS * 4], bf16).ap()
    sidx = nc.dram_tensor("sidx", [16, NT, K, WRAP], i16).ap()

    def mm(out, lhsT, rhs, start, stop):
        nc.tensor.matmul(out, lhsT, rhs, start=start, stop=stop)

    with tile.TileContext(nc) as tc, ExitStack() as ctx:
        wpool = ctx.enter_context(tc.tile_pool(name="weights", bufs=1))
        dpool = ctx.enter_context(tc.tile_pool(name="dp", bufs=1))
        xfctx = ExitStack()
        xfpool = xfctx.enter_context(tc.tile_pool(name="xfp", bufs=1))

        # ---- persistent weight tiles
        w1S = [wpool.tile([128, MID], bf16, tag=f"w1_{i}", name=f"w1_{i}") for i in range(4)]
        for i in range(4):
            nc.sync.dma_start(w1S[i][:], w1T[i * 128:(i + 1) * 128, :])
        axS = [wpool.tile([128, 2], bf16, tag=f"ax_{i}", name=f"ax_{i}") for i in range(2)]
        for i in range(2):
            nc.sync.dma_start(axS[i][:], axT[i * 128:(i + 1) * 128, :])
        ayS = [wpool.tile([128, 2], bf16, tag=f"ay_{i}", name=f"ay_{i}") for i in range(2)]
        for i in range(2):
            nc.sync.dma_start(ayS[i][:], ayT[i * 128:(i + 1) * 128, :])
        woffS = [wpool.tile([128, K, 18], bf16, tag=f"wo_{i}", name=f"wo_{i}") for i in range(2)]
        for i in range(2):
            nc.sync.dma_start(woffS[i][:], woffT[i * 128:(i + 1) * 128, :, :])
        w2S = [wpool.tile([128, OUT], bf16, tag=f"w2_{i}", name=f"w2_{i}") for i in range(18)]
        for i in range(18):
            nc.sync.dma_start(w2S[i][:], w2T[i * 128:(i + 1) * 128, :])
        b1S = [wpool.tile([128, 1], f32, tag=f"b1_{i}", name=f"b1_{i}") for i in range(2)]
        for i in range(2):
            nc.sync.dma_start(b1S[i][:], b1[i * 128:(i + 1) * 128, :])
        baS = wpool.tile([2, 1], f32, tag="ba")
        nc.sync.dma_start(baS[:], ba[:, :])
        b2S = [wpool.tile([128, 1], f32, tag=f"b2_{i}", name=f"b2_{i}") for i in range(2)]
        for i in range(2):
            nc.sync.dma_start(b2S[i][:], b2[i * 128:(i + 1) * 128, :])
        byS = wpool.tile([FR, TS], f32, tag="basey")
        nc.sync.dma_start(byS[:], basey[:, :])
        bxS = wpool.tile([FR, TS], f32, tag="basex")
        nc.sync.dma_start(bxS[:], basex[:, :])
        idm2S = wpool.tile([2, 2], f32, tag="idm2")
        nc.sync.dma_start(idm2S[:], idm2[:, :])

        # ---- padded bf16 xf storage, zeroed borders
        xfb = [xfpool.tile([128, XFBN], bf16, tag=f"xfb{cb}", name=f"xfb{cb}")
               for cb in range(2)]
        for cb in range(2):
            nc.vector.memset(xfb[cb][:], 0.0)

        # ================= phase A: conv1 + upsample + attention =========
        with tc.tile_pool(name="phA", bufs=3) as pa, \
             tc.tile_pool(name="phAy", bufs=1) as pay, \
             tc.tile_pool(name="psA", bufs=2, space="PSUM") as psA, \
             tc.tile_pool(name="psZ", bufs=2, space="PSUM") as psZ, \
             tc.tile_pool(name="psR", bufs=2, space="PSUM") as psR:
            ySr = [pay.tile([128, HY, WY], bf16, tag=f"y{i}", name=f"y{i}") for i in range(2)]
            for i in range(2):
                nc.sync.dma_start(
                    ySr[i][:], y_im[i * 128:(i + 1) * 128, :].rearrange(
                        "p (a b) -> p a b", a=HY, b=WY))
            xF = [pay.tile([128, P], bf16, tag=f"xF{i}", name=f"xF{i}") for i in range(4)]
            for i in range(4):
                for nt in range(NT):
                    nc.sync.dma_start(xF[i][:, nt * TS:(nt + 1) * TS],
                                      x_im[i * 128:(i + 1) * 128, nt * TS:(nt + 1) * TS])
            for nt in range(NT):
                xs = [xF[i][:, nt * TS:(nt + 1) * TS] for i in range(4)]
                # nearest-upsample via dup-read copies (2 per cb)
                yus = [pa.tile([128, 8, W], bf16, tag=f"yu{i}", name=f"yus{i}_{nt}")
                       for i in range(2)]
                for cb in range(2):
                    ysrc = ySr[cb][:, nt * 4:(nt + 1) * 4, :]
                    dup = ysrc.rearrange("p a (b o) -> p a b o", o=1).broadcast_to(
                        (128, 4, WY, 2))
                    for dy in range(2):
                        dst = yus[cb][:, dy::2, :].rearrange(
                            "p a (b o) -> p a b o", o=2)
                        nc.vector.tensor_copy(dst, dup)
                # conv1 + BN1 bias -> bf16
                xms = [pa.tile([128, TS], bf16, tag=f"xm{i}", name=f"xms{i}_{nt}")
                       for i in range(2)]
                for mb in range(2):
                    pt = psA.tile([128, TS], f32, tag="c1", name=f"c1_{mb}_{nt}")
                    for kt in range(4):
                        mm(pt[:], w1S[kt][:, mb * 128:(mb + 1) * 128], xs[kt],
                           kt == 0, kt == 3)
                    nc.scalar.activation(xms[mb][:], pt[:], AF.Identity,
                                         bias=b1S[mb][:])
                # attention logits (on xm + yu) + sigmoid
                pz = psZ.tile([2, TS], f32, tag="zp", name=f"zp_{nt}")
                for mb in range(2):
                    mm(pz[:], axS[mb][:], xms[mb][:], mb == 0, False)
                for cb in range(2):
                    yuf = yus[cb][:].rearrange("p a b -> p (a b)")
                    mm(pz[:], ayS[cb][:], yuf, False, cb == 1)
                zs = pa.tile([2, TS], f32, tag="zs", name=f"zs_{nt}")
                nc.scalar.activation(zs[:], pz[:], AF.Sigmoid, bias=baS[:])
                # replicate z rows across partitions via one-hot matmul
                zp = [psR.tile([128, TS], f32, tag="zr", name=f"zr{i}_{nt}")
                      for i in range(2)]
                for i in range(2):
                    mm(zp[i][:], idm2S[:, i:i + 1].broadcast_to((2, 128)), zs[:],
                       True, True)
                # xf = xm*z0 + yu*z1
                for cb in range(2):
                    yuf = yus[cb][:].rearrange("p a b -> p (a b)")
                    t0 = pa.tile([128, TS], f32, tag="t0", name=f"t0_{nt}_{cb}")
                    nc.vector.tensor_tensor(t0[:], xms[cb][:], zp[0][:], OP.mult)
                    t1 = pa.tile([128, TS], f32, tag="t1", name=f"t1_{nt}_{cb}")
                    nc.vector.tensor_tensor(t1[:], yuf, zp[1][:], OP.mult)
                    xff = pa.tile([128, TS], f32, tag="xff", name=f"xff_{nt}_{cb}")
                    nc.vector.tensor_tensor(xff[:], t0[:], t1[:], OP.add)
                    dstv = xfb[cb][:, :PP].rearrange("p (a b) -> p a b", a=HP, b=WP)
                    nc.vector.tensor_copy(
                        dstv[:, nt * 8 + 1:nt * 8 + 9, 1:57],
                        xff[:].rearrange("p (a b) -> p a b", a=8, b=W))
                    nc.sync.dma_start(
                        xf_im[cb * 128:(cb + 1) * 128, nt * TS:(nt + 1) * TS],
                        xff[:])

        # ================= phase B: offset conv ==========================
        with tc.tile_pool(name="phB", bufs=1) as pb, \
             tc.tile_pool(name="psB", bufs=2, space="PSUM") as psB:
            offS = pb.tile([18, P], f32, tag="off")
            for nt in range(NT):
                po = psB.tile([18, TS], f32, tag="offp", name=f"offp_{nt}")
                first = True
                for kk in range(K):
                    ky, kx = kk // 3, kk % 3
                    for cb in range(2):
                        rhs = xfb[cb][:, :PP].rearrange("p (a b) -> p a b", a=HP, b=WP)[
                            :, nt * 8 + ky:nt * 8 + ky + 8, kx:kx + W]
                        mm(po[:], woffS[cb][:, kk, :], rhs, first,
                           kk == K - 1 and cb == 1)
                        first = False
                nc.scalar.activation(offS[:, nt * TS:(nt + 1) * TS], po[:],
                                     AF.Identity)
            nc.sync.dma_start(soff[:, :], offS[:])

        # ================= phase C: index/weight pipeline (folded) =======
        with tc.tile_pool(name="phC", bufs=1) as pc:
            sofr = soff.rearrange("r (j s) -> r j s", j=FJ)

            def pp(nm, bufs=12):
                return pc.tile([FR, TS], f32, tag="pp", name=nm, bufs=bufs)

            offy = pp("offy")
            nc.sync.dma_start(offy[:], sofr[0:K].rearrange("r j s -> (r j) s"))
            offx = pp("offx")
            nc.sync.dma_start(offx[:], sofr[K:2 * K].rearrange("r j s -> (r j) s"))

            def floor_pipeline(offT, baseT, tag):
                s = pp(f"s_{tag}")
                nc.vector.tensor_tensor(s[:], offT[:], baseT[:], OP.add)
                ri = pc.tile([FR, TS], i32, tag="ppi", name=f"ri_{tag}", bufs=2)
                nc.vector.tensor_copy(ri[:], s[:])
                r0 = pp(f"r0_{tag}")
                nc.vector.tensor_copy(r0[:], ri[:])
                gt = pp(f"gt_{tag}")
                nc.vector.tensor_tensor(gt[:], r0[:], s[:], OP.is_gt)
                fl = pp(f"fl_{tag}")
                nc.vector.tensor_tensor(fl[:], r0[:], gt[:], OP.subtract)
                fr = pp(f"fr_{tag}")
                nc.vector.tensor_tensor(fr[:], s[:], fl[:], OP.subtract)
                # w0 = 1-fr ; w1 = fr*(fl>=-1) ; cl = clip(fl, -1, 56)
                w0 = pp(f"w0_{tag}")
                nc.vector.tensor_scalar(w0[:], fr[:], -1.0, 1.0, OP.mult, OP.add)
                m1 = pp(f"m1_{tag}")
                nc.vector.tensor_scalar(m1[:], fl[:], -1.0, None, OP.is_ge)
                w1 = pp(f"w1_{tag}")
                nc.vector.tensor_tensor(w1[:], fr[:], m1[:], OP.mult)
                cl = pp(f"cl_{tag}")
                nc.vector.tensor_scalar(cl[:], fl[:], -1.0, 56.0, OP.max, OP.min)
                return cl, w0, w1

            ycl, wy0, wy1 = floor_pipeline(offy, byS, "y")
            xcl, wx0, wx1 = floor_pipeline(offx, bxS, "x")

            # bilinear weight products [63, 448, 2(tb), 2(pair)] bf16
            wq = pc.tile([FR, TS, 2, 2], bf16, tag="wq")
            nc.vector.tensor_tensor(wq[:, :, 0, 0], wy0[:], wx0[:], OP.mult)
            nc.vector.tensor_tensor(wq[:, :, 0, 1], wy0[:], wx1[:], OP.mult)
            nc.vector.tensor_tensor(wq[:, :, 1, 0], wy1[:], wx0[:], OP.mult)
            nc.vector.tensor_tensor(wq[:, :, 1, 1], wy1[:], wx1[:], OP.mult)
            nc.sync.dma_start(wqd.rearrange("k j e -> (k j) e"),
                              wq[:].rearrange("r s t q -> r (s t q)"))

            # idx = (ycl+1)*58 + xcl + 1
            i0f = pp("i0f")
            nc.vector.tensor_scalar(i0f[:], ycl[:], 58.0, 59.0, OP.mult, OP.add)
            nc.vector.tensor_tensor(i0f[:], i0f[:], xcl[:], OP.add)
            ii = pc.tile([FR, TS], i32, tag="ppi", name="ii", bufs=2)
            nc.vector.tensor_copy(ii[:], i0f[:])
            is_ = pc.tile([FR, 16, WRAP], i16, tag="pps", name="is0")
            nc.vector.tensor_copy(
                is_[:].rearrange("r q s -> r s q"),
                ii[:].rearrange("r (s q) -> r s q", s=WRAP, q=16))
            for kk in range(K):
                nc.sync.dma_start(
                    sidx[:, :, kk, :].rearrange("q nt s -> nt q s"),
                    is_[kk * FJ:(kk + 1) * FJ, :, :])

        idxw = dpool.tile([128, NT, K, WRAP], i16, tag="idxw")
        for g in range(8):
            nc.sync.dma_start(idxw[g * 16:(g + 1) * 16], sidx[:, :, :, :])

        # gather source: per position, 8 packed bf16 =
        #   [cb0 (i, i+1), cb0 (i+58, i+59), cb1 (i, i+1), cb1 (i+58, i+59)]
        NB = XFBN - 60
        xq = dpool.tile([128, NB, 4], f32, tag="xq")
        xqb = xq[:].bitcast(bf16).rearrange("p n (c t q) -> p n c t q", c=2, t=2)
        # zero the tail so the shifted slot-1 copies can stop at NB-58
        nc.vector.memset(xq[:, NB - 72:NB, :], 0.0)
        for cb in range(2):
            nc.vector.tensor_copy(xqb[:, :NB, cb, 0, 0], xfb[cb][:, :NB])
            nc.vector.tensor_copy(xqb[:, :NB, cb, 0, 1], xfb[cb][:, 1:NB + 1])
        # slot (cb,1) at position i == slot (cb,0) at position i+58
        for cb in range(2):
            nc.vector.tensor_copy(xq[:, :NB - 58, cb * 2 + 1],
                                  xq[:, 58:NB, cb * 2])
        xfctx.close()  # release xfb

        # ================= phase D: gather + lerp + DCN matmul ===========
        with tc.tile_pool(name="wSp", bufs=1) as wsp, \
             tc.tile_pool(name="gtp", bufs=1) as gtp, \
             tc.tile_pool(name="v2p", bufs=1) as v2p, \
             tc.tile_pool(name="opool", bufs=2) as op_, \
             tc.tile_pool(name="psD", bufs=3, space="PSUM") as psD:
            for nt in range(NT):
                v2s = []
                for ky in range(3):
                    wS = wsp.tile([128, 3, TS, 2, 2], bf16, tag=f"wS{ky}",
                                  name=f"wS_{nt}_{ky}")
                    nc.sync.dma_start(
                        wS[:].rearrange("p k s t q -> p k (s t q)"),
                        wqd[3 * ky:3 * ky + 3, nt, :].rearrange(
                            "(o k) e -> o k e", o=1).broadcast_to(
                            (128, 3, TS * 4)))
                    gq = gtp.tile([128, 3, TS, 4], f32, tag=f"gq{ky}",
                                  name=f"gq_{nt}_{ky}")
                    idxs = idxw[:, nt, 3 * ky:3 * ky + 3, :]
                    nc.gpsimd.ap_gather(
                        gq[:], xq[:, 0:GN, :], idxs, channels=128,
                        num_elems=GN, d=4, num_idxs=3 * TS)
                    # in-place lerp: gq[*, cb, tb, pair] *= w[tb, pair], one op
                    g4 = gq[:].bitcast(bf16).rearrange(
                        "p a s (c t q) -> p (a s) c t q", c=2, t=2)
                    w8d = wS[:].rearrange(
                        "p (o a) s t q -> p (a s) o t q", o=1).broadcast_to(
                        (128, 3 * TS, 2, 2, 2))
                    nc.vector.tensor_tensor(g4[:, :, :, :, :], g4[:, :, :, :, :],
                                            w8d, OP.mult)
                    # tb-sum into V2 [128, 1344, 2cb, 2pair], one op
                    v2 = v2p.tile([128, 3, TS, 2, 2], bf16, tag=f"v2{ky}",
                                  name=f"v2_{nt}_{ky}")
                    g40 = gq[:].bitcast(bf16).rearrange(
                        "p a s (c t q) -> p (a s) c t q", c=2, t=2)[:, :, :, 0, :]
                    g41 = gq[:].bitcast(bf16).rearrange(
                        "p a s (c t q) -> p (a s) c t q", c=2, t=2)[:, :, :, 1, :]
                    nc.vector.tensor_tensor(
                        v2[:].rearrange("p a s c q -> p (a s) c q"), g40, g41, OP.add)
                    v2s.append(v2)
                # DCN: psum accumulate over (kk, cb, pair)
                for mb in range(2):
                    pD = psD.tile([128, TS], f32, tag="dcn", name=f"dcn_{nt}_{mb}")
                    first = True
                    for ky in range(3):
                        v2 = v2s[ky]
                        for kkr in range(3):
                            kk = 3 * ky + kkr
                            for cb in range(2):
                                lhsT = w2S[kk * 2 + cb][:, mb * 128:(mb + 1) * 128]
                                for pr in range(2):
                                    mm(pD[:], lhsT, v2[:, kkr, :, cb, pr], first,
                                       kk == K - 1 and cb == 1 and pr == 1)
                                    first = False
                    oS = op_.tile([128, TS], f32, tag="o", name=f"o_{nt}_{mb}")
                    nc.scalar.activation(oS[:], pD[:], AF.Identity, bias=b2S[mb][:])
                    nc.sync.dma_start(out_im[mb * 128:(mb + 1) * 128,
                                             nt * TS:(nt + 1) * TS], oS[:])

    nc.compile()
    return nc


def _in_maps(d):
    import ml_dtypes
    f = _fold_weights(d)
    byf, bxf = _build_consts()
    bf = ml_dtypes.bfloat16
    shared = {
        'w1T': f['w1T'].astype(bf), 'axT': f['axT'].astype(bf),
        'ayT': f['ayT'].astype(bf), 'woffT': f['woffT'].astype(bf),
        'w2T': f['w2T'].astype(bf),
        'b1': f['b1'], 'ba': f['ba'], 'b2': f['b2'],
        'basey': byf, 'basex': bxf,
        'idm2': np.eye(2, 2, dtype=np.float32),
    }
    maps = []
    for n in range(N):
        m = dict(shared)
        m['x_im'] = np.ascontiguousarray(d['x'][n].reshape(CIN, P)).astype(bf)
        m['y_im'] = np.ascontiguousarray(d['y'][n].reshape(MID, HY * WY)).astype(bf)
        maps.append(m)
    return maps


def kernel(**inputs):
    d = {k: np.asarray(v) for k, v in inputs.items()}
    if 'nc' not in _CACHE:
        _CACHE['nc'] = _build_program()
    nc = _CACHE['nc']
    from concourse.bass_utils import run_bass_kernel_spmd
    maps = _in_maps(d)
    res = run_bass_kernel_spmd(nc, maps, list(range(N)))
    _CACHE['last_res'] = res
    outs = np.stack([res.results[i]['out_im'].reshape(OUT, H, W) for i in range(N)])
    xfs = np.stack([res.results[i]['xf_im'].reshape(MID, H, W) for i in range(N)])
    return outs.astype(np.float32), xfs.astype(np.float32)
